# revision 3
# baseline (speedup 1.0000x reference)
"""Trainium2 Bass kernel for nn_DiarizationModel: 10 Adam iterations of
L1-basis fitting. T=50000 sharded over 8 cores; basis replicated.

Self-contained: hardcodes all shapes; host preps shards; device does the
10-iteration optimization; host transposes the tiny result back.
"""
import math
import numpy as np

import concourse.bass as bass
import concourse.bacc as bacc
import concourse.mybir as mybir
import concourse.tile as tile
from concourse.bass_utils import run_bass_kernel_spmd

F32 = mybir.dt.float32
F32R = mybir.dt.float32r
U32 = mybir.dt.uint32
I32 = mybir.dt.int32
AX = mybir.AxisListType
ALU = mybir.AluOpType
ACT = mybir.ActivationFunctionType

N_CORES = 8
D = 512
K = 16
T = 50000
SHARD = T // N_CORES        # 6250
NCH = 49                    # chunks of 128 T-rows per core (49*128 = 6272)
SHPAD = NCH * 128           # 6272
CW = D + K                  # 528 fused row width (negE | A)
FLATW = NCH * CW            # 25872

LAMBDA1 = 0.3366
LR = 0.1
CSH = LR * LAMBDA1          # shrink threshold
B1, B2, EPS = 0.9, 0.999, 1e-8
N_ITERS = 10
NEG_BIG = -1.0e30


def build_kernel():
    nc = bacc.Bacc(trn_type="TRN2", num_devices=N_CORES)

    get_d = nc.dram_tensor("get", [128, FLATW], F32R, kind="ExternalInput")
    ash_d = nc.dram_tensor("a_sh", [K, SHPAD], F32R, kind="ExternalInput")
    id_d = nc.dram_tensor("ident", [128, 128], F32R, kind="ExternalInput")
    pt_d = nc.dram_tensor("ptinit", [K, D], F32, kind="ExternalInput")
    jb_d = nc.dram_tensor("jbase", [1, 1], F32, kind="ExternalInput")
    pm_d = nc.dram_tensor("padmask", [128, 1], F32, kind="ExternalInput")
    io_d = nc.dram_tensor("iotap", [1, 128], F32, kind="ExternalInput")

    obt_d = nc.dram_tensor("out_bt", [K, D], F32, kind="ExternalOutput")
    dbg_d = nc.dram_tensor("out_dbg", [1, 32], F32, kind="ExternalOutput")

    get_flat = get_d[:].rearrange("p f -> () (p f)")

    with tile.TileContext(nc) as tc:
        with tc.tile_pool(name="per", bufs=1) as per, \
             tc.tile_pool(name="scr", bufs=2) as scr, \
             tc.tile_pool(name="pmain", bufs=2, space="PSUM") as pmain, \
             tc.tile_pool(name="ptail", bufs=3, space="PSUM") as ptail, \
             tc.tile_pool(name="dram", bufs=2, space="DRAM") as dram:

            # ---- persistent tiles ----
            get_sb = per.tile([128, FLATW], F32R)
            a_sb = per.tile([K, SHPAD], F32R)
            ident = per.tile([128, 128], F32R)
            padmask = per.tile([128, 1], F32)
            iotap = per.tile([1, 128], F32)
            jbase = per.tile([1, 1], F32)
            Ct = per.tile([128, 64], F32)
            PT = per.tile([K, D], F32)
            mT = per.tile([K, D], F32)
            vT = per.tile([K, D], F32)
            BT = per.tile([K, D], F32R)
            smult = per.tile([K, D], F32)
            sgnP = per.tile([K, D], F32)
            mask_l1 = per.tile([K, 1], F32)
            dbg = per.tile([1, 32], F32)

            idf = ident[:].bitcast(F32)

            # ---- loads ----
            NSLAB = 7
            for s in range(NSLAB):
                w = FLATW // NSLAB
                nc.sync.dma_start(get_sb[:, s * w:(s + 1) * w],
                                  get_d[:, s * w:(s + 1) * w])
            nc.sync.dma_start(a_sb[:], ash_d[:])
            nc.sync.dma_start(ident[:], id_d[:])
            nc.sync.dma_start(padmask[:], pm_d[:])
            nc.sync.dma_start(iotap[:], io_d[:])
            nc.sync.dma_start(jbase[:], jb_d[:])
            nc.sync.dma_start(PT[:], pt_d[:])

            nc.vector.memset(mT[:], 0.0)
            nc.vector.memset(vT[:], 0.0)
            nc.vector.memset(Ct[:], NEG_BIG)
            nc.vector.memset(dbg[:], 0.0)
            nc.vector.tensor_copy(BT[:], PT[:])         # iter-1 B = P (rounds)
            nc.scalar.sign(sgnP[:], PT[:])

            # PE warm-up touches: absorb DMA waits one at a time
            for s in range(NSLAB):
                wm = ptail.tile([1, 128], F32R, name="wm", tag="tail")
                nc.tensor.transpose(wm[:], get_sb[:, s * (FLATW // NSLAB):
                                                  s * (FLATW // NSLAB) + 1],
                                    ident[:])
            wm2 = ptail.tile([1, K], F32R, name="wm2", tag="tail")
            nc.tensor.transpose(wm2[:], a_sb[:, 0:1], ident[0:K, 0:K])
            gsem = nc.alloc_semaphore("gsem")
            gcnt = 0

            for t in range(1, N_ITERS + 1):
                c2t = 1.0 - B2 ** t
                kt = LR / (1.0 - B1 ** t)

                # ---------- main pass: colsums of |B A - E| ----------
                for grp in range(25):
                    ng = 2 if grp < 24 else 1
                    ps = pmain.tile([128, 1024], F32, name="ps", tag="mainps")
                    for q in range(ng):
                        ch = grp * 2 + q
                        o = ps[:, q * 512:(q + 1) * 512]
                        nc.tensor.matmul(o, ident[:],
                                         get_sb[:, ch * CW: ch * CW + D],
                                         start=True, stop=False)
                        nc.tensor.matmul(o, a_sb[:, ch * 128:(ch + 1) * 128],
                                         BT[:], start=False, stop=True)
                    view = ps[:, 0:ng * 512].rearrange("p (n d) -> p n d", d=512)
                    nc.vector.tensor_reduce(Ct[:, grp * 2: grp * 2 + ng], view,
                                            axis=AX.X, op=ALU.add,
                                            apply_absolute_value=True)

                # mask pad rows of last chunk
                nc.vector.tensor_scalar(Ct[:, 48:49], Ct[:, 48:49],
                                        padmask[:], None, op0=ALU.add)

                # ---------- local argmax ----------
                m8 = scr.tile([128, 8], F32, name="m8", tag="m8")
                i8 = scr.tile([128, 8], U32, name="i8", tag="i8")
                nc.vector.max(m8[:], Ct[:])
                nc.vector.max_index(i8[:], m8[:], Ct[:])
                stk = scr.tile([128, 2], F32, name="stk", tag="stk")
                nc.vector.tensor_copy(stk[:, 0:1], m8[:, 0:1])
                nc.vector.tensor_copy(stk[:, 1:2], i8[:, 0:1])
                tpm = ptail.tile([1, 128], F32, name="tpm", tag="tail")
                tpi = ptail.tile([1, 128], F32, name="tpi", tag="tail")
                nc.tensor.transpose(tpm[:], stk[:, 0:1], idf)
                nc.tensor.transpose(tpi[:], stk[:, 1:2], idf)
                rmx = scr.tile([1, 128], F32, name="rmx", tag="rmx")
                rix = scr.tile([1, 128], F32, name="rix", tag="rix")
                nc.scalar.copy(rmx[:], tpm[:])
                nc.scalar.copy(rix[:], tpi[:])
                g8 = scr.tile([1, 8], F32, name="g8", tag="g8")
                gi8 = scr.tile([1, 8], U32, name="gi8", tag="gi8")
                nc.vector.max(g8[:], rmx[:])
                nc.vector.max_index(gi8[:], g8[:], rmx[:])
                ploc = scr.tile([1, 1], F32, name="ploc", tag="ploc")
                nc.vector.tensor_copy(ploc[:], gi8[:, 0:1])
                # cloc = rix[ploc]
                eqm = scr.tile([1, 128], F32, name="eqm", tag="eqm")
                nc.vector.tensor_scalar(eqm[:], iotap[:], ploc[:], None,
                                        op0=ALU.is_equal)
                nc.vector.tensor_tensor(out=eqm[:], in0=eqm[:], in1=rix[:],
                                        op=ALU.mult)
                cloc = scr.tile([1, 1], F32, name="cloc", tag="cloc")
                nc.vector.tensor_reduce(cloc[:], eqm[:], axis=AX.X, op=ALU.add)
                # jglob = jbase + cloc*128 + ploc ; off = ploc*FLATW + cloc*CW
                jg = scr.tile([1, 1], F32, name="jg", tag="jg")
                nc.vector.scalar_tensor_tensor(jg[:], cloc[:], 128.0, ploc[:],
                                               op0=ALU.mult, op1=ALU.add)
                nc.vector.tensor_tensor(out=jg[:], in0=jg[:], in1=jbase[:],
                                        op=ALU.add)
                offf = scr.tile([1, 1], F32, name="offf", tag="offf")
                nc.vector.tensor_scalar_mul(offf[:], cloc[:], float(CW))
                nc.vector.scalar_tensor_tensor(offf[:], ploc[:], float(FLATW),
                                               offf[:], op0=ALU.mult, op1=ALU.add)
                offi = scr.tile([1, 1], I32, name="offi", tag="offi")
                nc.vector.tensor_copy(offi[:], offf[:])

                # ---------- candidate blob + AllGather ----------
                blob = scr.tile([1, 536], F32, name="blob", tag="blob")
                nc.scalar.copy(blob[:, 0:1], g8[:, 0:1])
                nc.scalar.copy(blob[:, 1:2], jg[:])
                with tc.tile_critical():
                    offv = nc.gpsimd.value_load(offi[:])
                    gcnt += 16
                    nc.gpsimd.dma_start(
                        blob[:, 2:2 + CW],
                        get_flat[0:1, bass.ds(offv, CW)]).then_inc(gsem, 16)
                    nc.gpsimd.wait_ge(gsem, gcnt)
                agi = dram.tile([1, 536], F32, name="agi", tag="agi")
                ago = dram.tile([8, 536], F32, name="ago", tag="ago",
                                addr_space="Shared")
                nc.sync.dma_start(agi[:], blob[:])
                nc.gpsimd.collective_compute(
                    "AllGather", ALU.bypass,
                    replica_groups=[list(range(N_CORES))],
                    ins=[agi[:]], outs=[ago[:]])

                # ---------- winner ----------
                vals8 = scr.tile([1, 8], F32, name="vals8", tag="vals8")
                with nc.allow_non_contiguous_dma(reason="8-elem maxval gather"):
                    nc.sync.dma_start(vals8[:], ago[:, 0:1].transpose([1, 0]))
                w8 = scr.tile([1, 8], F32, name="w8", tag="w8")
                wi8 = scr.tile([1, 8], U32, name="wi8", tag="wi8")
                nc.vector.max(w8[:], vals8[:])
                nc.vector.max_index(wi8[:], w8[:], vals8[:])
                winner = scr.tile([1, 1 + CW], F32, name="winner", tag="winner")
                with tc.tile_critical():
                    wv = nc.gpsimd.value_load(wi8[0:1, 0:1])
                    gcnt += 16
                    nc.gpsimd.dma_start(
                        winner[:], ago[bass.ds(wv, 1), 1:2 + CW]).then_inc(gsem, 16)
                    nc.gpsimd.wait_ge(gsem, gcnt)

                # debug capture
                if t <= 10:
                    nc.scalar.copy(dbg[:, t - 1:t], winner[:, 0:1])
                    nc.scalar.copy(dbg[:, 9 + t:10 + t], w8[:, 0:1])

                # ---------- gradient ----------
                # a column (16,1) f32r via PE transpose of winner[513:529]
                acT = ptail.tile([K, 1], F32, name="acT", tag="tail")
                nc.tensor.transpose(acT[:], winner[:, 1 + D:1 + D + K],
                                    idf[0:1, 0:1])
                acol = scr.tile([K, 1], F32R, name="acol", tag="acol")
                nc.scalar.copy(acol[:], acT[:])
                ba = ptail.tile([1, D], F32, name="ba", tag="tail")
                nc.tensor.matmul(ba[:], acol[:], BT[:], start=True, stop=True)
                u = scr.tile([1, D], F32, name="u", tag="u")
                nc.vector.tensor_tensor(out=u[:], in0=winner[:, 1:1 + D],
                                        in1=ba[:], op=ALU.add)
                srow = scr.tile([1, D], F32R, name="srow", tag="srow")
                nc.scalar.sign(srow[:], u[:])
                arow = scr.tile([1, K], F32R, name="arow", tag="arow")
                nc.vector.tensor_copy(arow[:], winner[:, 1 + D:1 + D + K])
                g1T = ptail.tile([K, D], F32, name="g1T", tag="tail")
                nc.tensor.matmul(g1T[:], arow[:], srow[:], start=True, stop=True)

                # g2: k* = argmax colsum|B|  (BT bytes; sign(B)=sgnP)
                cb = scr.tile([K, 1], F32, name="cb", tag="cb")
                nc.vector.tensor_reduce(cb[:], BT[:].bitcast(F32), axis=AX.X,
                                        op=ALU.add, apply_absolute_value=True)
                cbT = ptail.tile([1, K], F32, name="cbT", tag="tail")
                nc.tensor.transpose(cbT[:], cb[:], idf[0:K, 0:K])
                rcb = scr.tile([1, K], F32, name="rcb", tag="rcb")
                nc.scalar.copy(rcb[:], cbT[:])
                cb8 = scr.tile([1, 8], F32, name="cb8", tag="cb8")
                nc.vector.max(cb8[:], rcb[:])
                kmr = scr.tile([1, K], F32, name="kmr", tag="kmr")
                nc.vector.tensor_scalar(kmr[:], rcb[:], cb8[:, 0:1], None,
                                        op0=ALU.is_ge)
                kmT = ptail.tile([K, 1], F32, name="kmT", tag="tail")
                nc.tensor.transpose(kmT[:], kmr[:], idf[0:1, 0:1])
                nc.scalar.mul(mask_l1[:], kmT[:], LAMBDA1)

                gT = scr.tile([K, D], F32, name="gT", tag="gT")
                nc.vector.scalar_tensor_tensor(gT[:], sgnP[:], mask_l1[:],
                                               g1T[:], op0=ALU.mult, op1=ALU.add)
                if t >= 2:
                    nc.vector.tensor_tensor(out=gT[:], in0=gT[:], in1=smult[:],
                                            op=ALU.mult)

                # ---------- Adam ----------
                sq = scr.tile([K, D], F32, name="sq", tag="sq")
                nc.scalar.activation(sq[:], gT[:], ACT.Square,
                                     scale=math.sqrt(1.0 - B2))
                gs = scr.tile([K, D], F32, name="gs", tag="gs")
                nc.scalar.mul(gs[:], gT[:], 1.0 - B1)
                nc.vector.scalar_tensor_tensor(vT[:], vT[:], B2, sq[:],
                                               op0=ALU.mult, op1=ALU.add)
                nc.vector.scalar_tensor_tensor(mT[:], mT[:], B1, gs[:],
                                               op0=ALU.mult, op1=ALU.add)
                dn = scr.tile([K, D], F32, name="dn", tag="dn")
                nc.scalar.activation(dn[:], vT[:], ACT.Sqrt, scale=1.0 / c2t)
                nc.vector.tensor_scalar_add(dn[:], dn[:], EPS)
                nc.vector.reciprocal(dn[:], dn[:])
                nc.vector.tensor_tensor(out=dn[:], in0=mT[:], in1=dn[:],
                                        op=ALU.mult)
                nc.vector.scalar_tensor_tensor(PT[:], dn[:], -kt, PT[:],
                                               op0=ALU.mult, op1=ALU.add)

                # ---------- shrink / next-iteration B ----------
                t1 = scr.tile([K, D], F32, name="t1", tag="t1")
                nc.vector.tensor_scalar_sub(t1[:], PT[:], CSH)
                qq = scr.tile([K, D], F32, name="qq", tag="qq")
                nc.vector.tensor_tensor(out=qq[:], in0=PT[:], in1=t1[:],
                                        op=ALU.mult)
                if t < N_ITERS:
                    nc.scalar.sign(smult[:], qq[:])
                    nc.vector.tensor_tensor(out=BT[:], in0=smult[:],
                                            in1=t1[:], op=ALU.mult)
                    nc.scalar.sign(sgnP[:], PT[:])
                else:
                    sf = scr.tile([K, D], F32, name="sf", tag="sf")
                    nc.scalar.sign(sf[:], qq[:])
                    btf = scr.tile([K, D], F32, name="btf", tag="btf")
                    nc.vector.tensor_tensor(out=btf[:], in0=sf[:], in1=t1[:],
                                            op=ALU.mult)
                    nc.sync.dma_start(obt_d[:], btf[:])
                    nc.sync.dma_start(dbg_d[:], dbg[:])

    nc.compile()
    return nc


_CACHE = {}


def _prep_inputs(embedding, basis_init, activation_init):
    E = np.ascontiguousarray(embedding, dtype=np.float32)
    A = np.ascontiguousarray(activation_init, dtype=np.float32)
    B0 = np.ascontiguousarray(basis_init, dtype=np.float32)
    ET = np.ascontiguousarray(E.T)              # (T, D)
    ident = np.eye(128, dtype=np.float32)
    ptinit = np.ascontiguousarray(B0.T)         # (K, D)
    padmask = np.zeros((128, 1), np.float32)
    padmask[SHARD - 48 * 128:, :] = NEG_BIG     # rows 106.. of chunk 48
    iotap = np.arange(128, dtype=np.float32).reshape(1, 128)

    in_maps = []
    for c in range(N_CORES):
        lo = c * SHARD
        slabE = np.zeros((SHPAD, D), np.float32)
        slabE[:SHARD] = -ET[lo:lo + SHARD]
        slabA = np.zeros((SHPAD, K), np.float32)
        slabA[:SHARD] = A[:, lo:lo + SHARD].T
        fused = np.concatenate([slabE, slabA], axis=1)        # (SHPAD, CW)
        get = np.ascontiguousarray(
            fused.reshape(NCH, 128, CW).transpose(1, 0, 2).reshape(128, FLATW))
        ash = np.zeros((K, SHPAD), np.float32)
        ash[:, :SHARD] = A[:, lo:lo + SHARD]
        in_maps.append({
            "get": get,
            "a_sh": ash,
            "ident": ident,
            "ptinit": ptinit,
            "jbase": np.array([[float(lo)]], np.float32),
            "padmask": padmask,
            "iotap": iotap,
        })
    return in_maps


def kernel(embedding, basis_init, activation_init, k, _want_debug=False):
    if "nc" not in _CACHE:
        _CACHE["nc"] = build_kernel()
    nc = _CACHE["nc"]
    in_maps = _prep_inputs(embedding, basis_init, activation_init)
    res = run_bass_kernel_spmd(nc, in_maps, core_ids=list(range(N_CORES)),
                               trace=_CACHE.get("trace", False))
    r0 = res.results[0]
    out_b = np.ascontiguousarray(r0["out_bt"].T)      # (D, K)
    out_a = np.asarray(activation_init, dtype=np.float32)
    _CACHE["last_res"] = res
    _CACHE["last_dbg"] = r0["out_dbg"]
    if _want_debug:
        return (out_b, out_a), r0["out_dbg"]
    return out_b, out_a
